# revision 8
# baseline (speedup 1.0000x reference)
"""Trainium2 Bass kernel for nn_CustomLoss (CrossEntropy + binary-remap BCE).

loss = mean_i[ logsumexp(pred_i) - pred_i[t_i] ]
     + 100 * mean_i[ 1{ LUT[argmax(pred_i)] != LUT[t_i] } ]

with LUT = [0,0,1,1,1,1,1,1,0,0]  (LUT[j] = 1 iff 2 <= j <= 7).

Sharding: data-parallel over the batch axis across 8 NeuronCores.  The host
CLASS-BUCKETS the rows: all rows with target class PERM[b] land in bucket b
(cols [200b, 200b+200)) of the per-core [128, 10, 2000] class-major layout,
with classes reordered as PERM = [2..7, 0, 1, 8, 9] so that

  * mid-6 classes = rows 0:6, outer-4 = rows 6:10 (contiguous max trees),
  * bucket b's target logit is row b (x_t sums = plain strided reduces),
  * bt = LUT[target] is 1 for cols [0, 1200) and 0 for [1200, 2000), so
    tiles [200, 1000, 600, 200] are bt-uniform and the BCE mismatch count
    is sum(sign(+-(m6 - m4))) on the Scalar engine -> exact bf16-tie
    half-weighting: mism = (Sum sign + N)/2.

Bucketing is a pure row permutation plus all-zero pad rows (corrections are
exact and host-known).  Everything streams bf16.  Per-engine (per core):
  DMA   : 4 tiles [128, 10*w] bf16, 5.12 MB total; small first tile so the
          ACT/DVE pipeline starts ~3 us earlier
  ACT   : exp per tile, sign(d) per tile, 2 ln+accum over the concatenated
          row-sum buffer s_all [128, 2000]
  DVE   : bf16 2x tensor_tensor trees (row-sum 5+5->2+2+1->1, max trees),
          d = +-(m6 - m4), 10 strided bucket-column reduces
  GPSIMD/PE: idle

Emission is phase-split (all DMA+exp first, then per-tile tail work) with
per-tile dedicated buffers so no tile serializes behind another.
"""

import numpy as np

# ---------------------------------------------------------------- constants
N = 2_000_000
C = 10
N_CORES = 8
P = 128
BUCKET_COLS = 200                 # per-class bucket width (cols per partition)
W_CORE = BUCKET_COLS * C          # 2000
ROWS_CORE_PAD = P * W_CORE        # 256,000 rows per core incl. pads
PERM = [2, 3, 4, 5, 6, 7, 0, 1, 8, 9]   # class of row r / bucket b
TILE_WS = [200, 1000, 600, 200]
TILE_BT = [1, 1, 0, 0]            # LUT[PERM[b]] per tile (uniform by design)
N_TILES = len(TILE_WS)
N_PADS = N_CORES * ROWS_CORE_PAD - N  # 48,000 all-zero pad rows

_CACHE = {}


# ------------------------------------------------------------- device build
def _build_nc():
    import concourse.tile as tile
    from concourse import bacc, mybir

    f32 = mybir.dt.float32
    bf16 = mybir.dt.bfloat16
    A = mybir.ActivationFunctionType
    X = mybir.AxisListType.X
    alu = mybir.AluOpType

    nc = bacc.Bacc("TRN2", target_bir_lowering=False, debug=False,
                   num_devices=N_CORES)
    comb_ds = [
        nc.dram_tensor(f"comb{i}", [P, wi * C], bf16,
                       kind="ExternalInput").ap()
        for i, wi in enumerate(TILE_WS)
    ]
    out_d = nc.dram_tensor("out", [P, 4], f32, kind="ExternalOutput").ap()

    with tile.TileContext(nc) as tc:
        with (
            tc.tile_pool(name="io", bufs=1) as io,
            tc.tile_pool(name="ep", bufs=1) as ep,
            tc.tile_pool(name="wp", bufs=1) as wp,
            tc.tile_pool(name="cp", bufs=1) as cp,
        ):
            acc_b = cp.tile([P, C], f32)          # per-bucket x_t sums
            acc_sg = cp.tile([P, N_TILES], f32)   # per-tile sign sums
            acc_ln = cp.tile([P, 2], f32)         # two ln accum slots
            s_all = cp.tile([P, W_CORE], bf16)    # concatenated row-sums

            # ---- phase A: all DMAs + exps
            cvs, ets = [], []
            for i, w in enumerate(TILE_WS):
                ct = io.tile([P, C * w], bf16, tag=f"comb{i}")
                nc.sync.dma_start(ct[:], comb_ds[i])
                cv = ct[:].rearrange("p (c w) -> p c w", c=C)
                et = ep.tile([P, C, w], bf16, tag=f"E{i}")
                nc.scalar.activation(et[:], cv, A.Exp)
                cvs.append(cv)
                ets.append(et)

            # ---- phase B: per-tile max trees, sign, sum tree, bucket sums
            col = 0
            bkt = 0
            for i, w in enumerate(TILE_WS):
                cv, et = cvs[i], ets[i]

                m1 = wp.tile([P, 3, w], bf16, tag=f"m1_{i}")
                nc.vector.tensor_tensor(m1[:], cv[:, 0:3, :], cv[:, 3:6, :],
                                        op=alu.max)
                m2 = wp.tile([P, w], bf16, tag=f"m2_{i}")
                nc.vector.tensor_tensor(m2[:], m1[:, 0, :], m1[:, 1, :],
                                        op=alu.max)
                m6 = wp.tile([P, w], bf16, tag=f"m6_{i}")
                nc.vector.tensor_tensor(m6[:], m2[:], m1[:, 2, :], op=alu.max)
                o1 = wp.tile([P, 2, w], bf16, tag=f"o1_{i}")
                nc.vector.tensor_tensor(o1[:], cv[:, 6:8, :], cv[:, 8:10, :],
                                        op=alu.max)
                m4 = wp.tile([P, w], bf16, tag=f"m4_{i}")
                nc.vector.tensor_tensor(m4[:], o1[:, 0, :], o1[:, 1, :],
                                        op=alu.max)
                # orient so sum(sign(d)) counts mismatches positively:
                # bt=1: mismatch iff m4 > m6 ; bt=0: mismatch iff m6 > m4
                da, db = (m4, m6) if TILE_BT[i] == 1 else (m6, m4)
                d = wp.tile([P, w], bf16, tag=f"d_{i}")
                nc.vector.tensor_tensor(d[:], da[:], db[:], op=alu.subtract)
                sg = wp.tile([P, w], bf16, tag=f"sg_{i}")
                nc.scalar.activation(sg[:], d[:], A.Sign,
                                     accum_out=acc_sg[:, i:i + 1])

                # CE row-sum tree -> s_all slice
                l1 = wp.tile([P, 5, w], bf16, tag=f"l1_{i}")
                nc.vector.tensor_tensor(l1[:], et[:, 0:5, :], et[:, 5:10, :],
                                        op=alu.add)
                l2 = wp.tile([P, 2, w], bf16, tag=f"l2_{i}")
                nc.vector.tensor_tensor(l2[:], l1[:, 0:2, :], l1[:, 2:4, :],
                                        op=alu.add)
                l3 = wp.tile([P, w], bf16, tag=f"l3_{i}")
                nc.vector.tensor_tensor(l3[:], l2[:, 0, :], l2[:, 1, :],
                                        op=alu.add)
                nc.vector.tensor_tensor(s_all[:, col:col + w], l3[:],
                                        l1[:, 4, :], op=alu.add)

                # CE gather: per-bucket target-column sums (row b = bucket b)
                nb = w // BUCKET_COLS
                for j in range(nb):
                    b = bkt + j
                    lo = j * BUCKET_COLS
                    nc.vector.reduce_sum(acc_b[:, b:b + 1],
                                         cv[:, b, lo:lo + BUCKET_COLS],
                                         axis=X)
                bkt += nb

                # ln over the first 1200 cols as soon as tiles 0-1 are done
                if i == 1:
                    lns0 = wp.tile([P, 1200], bf16, tag="lns0")
                    nc.scalar.activation(lns0[:], s_all[:, 0:1200], A.Ln,
                                         accum_out=acc_ln[:, 0:1])
                col += w

            lns1 = wp.tile([P, 800], bf16, tag="lns1")
            nc.scalar.activation(lns1[:], s_all[:, 1200:2000], A.Ln,
                                 accum_out=acc_ln[:, 1:2])

            # ---- final per-partition fold + store
            out_t = cp.tile([P, 4], f32)
            nc.vector.reduce_sum(out_t[:, 0:1], acc_ln[:], axis=X)
            nc.vector.reduce_sum(out_t[:, 1:2], acc_b[:], axis=X)
            nc.vector.reduce_sum(out_t[:, 2:3], acc_sg[:], axis=X)
            nc.vector.tensor_copy(out_t[:, 3:4], acc_ln[:, 0:1])
            nc.sync.dma_start(out_d[:], out_t[:])

    # Force a single activation table containing Exp+Ln+Sign so the
    # compiler does not ping-pong ACT_TABLE_LOADs.
    import concourse.bacc as bacc_mod
    from concourse.hw_specs import get_activation_tables
    orig = get_activation_tables(nc.m.arch)
    combined = None
    for k, v in orig.items():
        if (mybir.ActivationFunctionType.Exp in v
                and mybir.ActivationFunctionType.Ln in v
                and mybir.ActivationFunctionType.Sign in v):
            combined = k
            break
    if combined is not None:
        patched = {k: (v if k == combined else set()) for k, v in orig.items()}
        saved = bacc_mod.get_activation_tables
        bacc_mod.get_activation_tables = lambda arch: patched
        try:
            nc.compile()
        finally:
            bacc_mod.get_activation_tables = saved
    else:
        nc.compile()
    return nc


def _get_nc():
    if "nc" not in _CACHE:
        _CACHE["nc"] = _build_nc()
    return _CACHE["nc"]


# ------------------------------------------------------------------- host
def _host_prep(pred, target):
    """Class-bucketed shard/pack: bf16 tiles [P, 10, w_i] per core."""
    import ml_dtypes

    pred = np.asarray(pred)
    if pred.dtype != ml_dtypes.bfloat16:
        pred = pred.astype(np.float32).astype(ml_dtypes.bfloat16)
    pred = pred[:, PERM]              # class perm: row r holds class PERM[r]
    target = np.asarray(target).astype(np.int32)
    # bucket index of each row: inverse perm of its target class
    inv = np.empty(C, np.int64)
    inv[np.asarray(PERM)] = np.arange(C)
    tb = inv[target]

    order = np.argsort(tb, kind="stable")
    counts = np.bincount(tb, minlength=C)
    offs = np.zeros(C + 1, np.int64)
    offs[1:] = np.cumsum(counts)

    tile_cols = np.cumsum([0] + TILE_WS)
    in_maps = []
    for k in range(N_CORES):
        R = np.full((C, BUCKET_COLS * P), -1, np.int64)
        for b in range(C):
            cnt = int(counts[b])
            base, rem = divmod(cnt, N_CORES)
            share = base + (1 if k < rem else 0)
            assert share <= BUCKET_COLS * P, (
                f"bucket {b} overflow on core {k}: {share}")
            start = offs[b] + k * base + min(k, rem)
            R[b, :share] = order[start:start + share]
        # [C, P*200] -> [C, P, 200] -> [P, C, 200] -> [P, W_CORE]
        Rpw = R.reshape(C, P, BUCKET_COLS).transpose(1, 0, 2)

        flat = Rpw.reshape(-1)
        Xg = pred[np.where(flat >= 0, flat, 0)]
        Xg[flat < 0] = ml_dtypes.bfloat16(0.0)
        # [P, C_bucket, 200, C_row] -> [P, C_row, C_bucket*200]
        Xc = Xg.reshape(P, C, BUCKET_COLS, C).transpose(0, 3, 1, 2) \
               .reshape(P, C, W_CORE)

        m = {}
        for i, w in enumerate(TILE_WS):
            sl = Xc[:, :, tile_cols[i]:tile_cols[i + 1]]
            m[f"comb{i}"] = np.ascontiguousarray(sl).reshape(P, C * w)
        in_maps.append(m)
    return in_maps


def kernel(pred, target):
    from concourse.bass_utils import run_bass_kernel_spmd

    nc = _get_nc()
    in_maps = _host_prep(pred, target)
    res = run_bass_kernel_spmd(nc, in_maps, core_ids=list(range(N_CORES)))

    s_ln = s_xt = s_sg = 0.0
    for k in range(N_CORES):
        o = res.results[k]["out"].astype(np.float64)
        s_ln += o[:, 0].sum()
        s_xt += o[:, 1].sum()
        s_sg += o[:, 2].sum()

    # all-zero pad rows: s = 10 -> ln(10); x_t col adds 0; d = 0 -> sign 0,
    # so each pad adds 0.5 to (S + L)/2 -- cancelled by using N not L below.
    s_ln -= N_PADS * np.log(10.0)
    mism = 0.5 * s_sg + 0.5 * N

    ce = (s_ln - s_xt) / N
    bce = 100.0 * mism / N
    return np.float32(ce + bce)
